# revision 1
# baseline (speedup 1.0000x reference)
"""CaLCS (soft-LCS) loss kernel for Trainium2, 8 NeuronCores, data-parallel
over batch.

Cost model of this target (measured): unrolled instructions cost ~20-25us
each (instruction fetch); instructions inside a For_i hardware loop are
cached and cost ~1-3us + ~1-4ns/element. A DVE op whose output stream lands
in the same 64-byte SBUF phase as one of its input streams runs ~9x slower,
so scan outputs are deliberately offset 4B from their d0/d1 reads.

Math (per core, one example; probs = softmax over V=32000):
  p[i,j] = probs[i, docs[j]] (cols pre-gathered on host), q = 1-p
  D[i,j] = p*(1+D[i-1,j-1]) + q*max(D[i-1,j], D[i,j-1])
  loss   = -log(mean_b min(D[511,511], 100) / 512)
Normalized alpha_j = D[i,j]/R_j with R_j = prod_{t<=j} q_t turns the row
recurrence into an interleaved (max,+) scan:
  even step: s = max(v_j, s) + khat_j ; odd step: s = s + pinv_j
with khat_j = alpha'_{j-1}*F_j, v_j = alpha'_j*E_j,
F_j = R'_{j-1}*pinv_j, E_j = R'_j/R_{j-1}, pinv_j = p_j/R_j.

Phase 1 (fully static, all 4 row-groups batched width-wise): stream logits
(Pool DMA queue) with exp+row-sum on ACT; build E/F/pinv with a handful of
[128, 4*512]-wide ops; pack DRAM rows of 2048 f32:
[F(512), E(512), d1(1024, odd=pinv)].
Phase 2 (For_i, 32 iters x 16 rows): two-half double-buffered staging DMA
(dynamic DRAM offsets, SP queue) + per row one product tensor_tensor
(writes d1 even = khat, d0 even = v) and one 1024-elem interleaved scan,
all on DVE. d0 odd slots (-BIG) live only in SBUF, set once.
Host: gathers 8 clamped D values, returns -log(mean/512).
"""

import numpy as np

import bass_rust
import concourse.bass as bass
import concourse.tile as tile
import concourse.mybir as mybir
from concourse import bass_utils

# ---- problem constants (hardcoded per contract) ----
B = 8
R = 512          # generation steps (rows of DP grid)
V = 32000        # vocab
C = 512          # doc length (cols of DP grid)
CLAMP = 100.0
P = 128          # SBUF partitions
NGRP = R // P    # 4 row groups
VCHUNK = 16000
NCHUNK = V // VCHUNK   # 2 chunks per row group
FW = C
DROW = 4 * FW    # packed DRAM row: F, E, d1(1024)  = 2048 f32
SLOT = 6 * FW    # SBUF slot: F, E, d1(1024), d0(1024) = 3072 f32
U = 8            # rows per half-body; body = 2U rows
NIT = R // (2 * U)
CE = C + 1       # R-vector row width (with leading 1)
NEGBIG = -1.0e30

F32 = mybir.dt.float32
BF16 = mybir.dt.bfloat16
ALU = mybir.AluOpType
ACTF = mybir.ActivationFunctionType


def _patched_drain_and_barrier(self, tick_clock, wait_clock):
    """Split the kernel-tail drain's sem waits across multiple drain
    instructions — core_v3 codegen rejects multi-wait CTRL instructions."""
    from concourse.tile import ScopedClock

    nc = self.nc
    probe = nc.sync.drain()
    wait_clock.add_sem_waits(probe.ins, ScopedClock({None: tick_clock.global_clock}))
    waits = list(probe.ins.sync_info.on_wait) if probe.ins.sync_info else []
    if len(waits) > 1:
        probe.ins.sync_info = bass_rust.SyncInfo(on_wait=waits[:1], on_update=[])
        for i in range(1, len(waits)):
            d = nc.sync.drain()
            d.ins.sync_info = bass_rust.SyncInfo(on_wait=[waits[i]], on_update=[])
    nc.all_engine_barrier()
    popped = nc._tile_sem_poison_stack.pop()
    assert popped is self._sem_poison
    nc.clear_and_free_semaphores(list(self.sems.allocated().values()))
    nc.all_engine_barrier()


tile.TileContext._drain_and_barrier = _patched_drain_and_barrier


def _split_multi_waits(nc: bass.Bass):
    """Walrus codegen for TRN2 accepts at most one sem wait per instruction.
    Hoist extra waits into same-engine NoOp/Drain instructions inserted
    immediately before the offending instruction."""
    n_split = 0
    for fn in nc.m.functions:
        for blk in fn.blocks:
            il = blk.instructions
            i = 0
            while i < len(il):
                inst = il[i]
                si = inst.sync_info
                if si is not None and len(si.on_wait) > 1:
                    waits = list(si.on_wait)
                    inst.sync_info = bass_rust.SyncInfo(
                        on_wait=[waits[0]], on_update=list(si.on_update)
                    )
                    for k, w in enumerate(waits[1:]):
                        if inst.engine == mybir.EngineType.PE:
                            filler = mybir.InstDrain(
                                name=f"wsplit-{inst.name}-{k}", engine=inst.engine,
                                sync_info=bass_rust.SyncInfo(on_wait=[w], on_update=[]),
                            )
                        else:
                            filler = mybir.InstNoOp(
                                name=f"wsplit-{inst.name}-{k}", engine=inst.engine,
                                sync_info=bass_rust.SyncInfo(on_wait=[w], on_update=[]),
                            )
                        il.insert(i, filler)
                        i += 1
                        n_split += 1
                i += 1
    return n_split


def build_nc(timing_reps: int = 0, *,
             do_phase1: bool = True, do_dp: bool = True) -> bass.Bass:
    """timing_reps=0: real kernel (external inputs). timing_reps=K>0:
    inputs are Internal (device zero-filled) and the body repeats K times
    with barriers; the wall-clock slope between rep counts isolates
    per-invocation device time."""
    nc = bass.Bass(trn_type="TRN2")
    kind = "Internal" if timing_reps else "ExternalInput"
    x = nc.dram_tensor("x", [R, V], BF16, kind=kind)
    cols = nc.dram_tensor("cols", [R, C], F32, kind=kind)
    out = nc.dram_tensor("out", [1, 1], F32, kind="ExternalOutput")
    packed = nc.dram_tensor("packed", [(R + 2 * U) * DROW], F32,
                            kind="Internal")

    with tile.TileContext(nc) as tc:
        with tc.tile_pool(name="keep", bufs=1) as keep:
            rlast = keep.tile([1, 64], F32, tag="rlast", name="rlast")
            if timing_reps:
                with tc.tile_pool(name="zpool", bufs=1) as zpool:
                    zx = zpool.tile([P, VCHUNK], BF16, tag="zx")
                    nc.vector.memset(zx[:, :], 0.0)
                    zxf = zpool.tile([P, C], F32, tag="zxf")
                    nc.vector.memset(zxf[:, :], 0.0)
                    for grp in range(NGRP):
                        for k in range(NCHUNK):
                            nc.gpsimd.dma_start(
                                out=x[grp * P:(grp + 1) * P,
                                      k * VCHUNK:(k + 1) * VCHUNK],
                                in_=zx[:, :])
                        nc.gpsimd.dma_start(
                            out=cols[grp * P:(grp + 1) * P, :], in_=zxf[:, :])
                tc.strict_bb_all_engine_barrier()

            def emit_phase1():
                with (
                    tc.tile_pool(name="chunks", bufs=2) as chunks,
                    tc.tile_pool(name="p1", bufs=1) as p1,
                ):
                    ones = p1.tile([P, C], F32, tag="ones")
                    nc.gpsimd.memset(ones[:, :], 1.0)
                    cols4 = p1.tile([P, NGRP * C], F32, tag="cols4",
                                    name="cols4")
                    ct4 = p1.tile([P, NGRP * C], F32, tag="ct4", name="ct4")
                    qt4 = p1.tile([P, NGRP * C], F32, tag="qt4", name="qt4")
                    fe8 = p1.tile([P, NGRP * 2 * C], F32, tag="fe8",
                                  name="fe8")
                    sums = p1.tile([P, 16], F32, tag="sums", name="sums")
                    z4 = p1.tile([P, 16], F32, tag="z4", name="z4")
                    rcp4 = p1.tile([P, 16], F32, tag="rcp4", name="rcp4")
                    d1img = p1.tile([P, NGRP * 2 * C], F32, tag="d1img",
                                    name="d1img")
                    rext4 = p1.tile([P, NGRP * CE], F32, tag="rext4",
                                    name="rext4")
                    irext4 = p1.tile([P, NGRP * CE], F32, tag="irext4",
                                     name="irext4")
                    rp4 = p1.tile([P, NGRP * CE], F32, tag="rp4", name="rp4")

                    # cols into SBUF (static, SP queue)
                    for g in range(NGRP):
                        nc.sync.dma_start(
                            out=cols4[:, g * C:(g + 1) * C],
                            in_=cols[g * P:(g + 1) * P, :])
                    # exp+row-sum stream (static, Pool queue + ACT)
                    for g in range(NGRP):
                        for k in range(NCHUNK):
                            t = chunks.tile([P, VCHUNK], BF16, tag="stream",
                                            name="stream_t")
                            nc.gpsimd.dma_start(
                                out=t[:, :],
                                in_=x[g * P:(g + 1) * P,
                                      k * VCHUNK:(k + 1) * VCHUNK])
                            nc.scalar.activation(
                                out=t[:, :], in_=t[:, :], func=ACTF.Exp,
                                accum_out=sums[:, g * NCHUNK + k:
                                               g * NCHUNK + k + 1])
                    # z = per-group sum; rcp = 1/z
                    nc.vector.tensor_tensor(
                        out=z4[:, 0:NGRP],
                        in0=bass.AP(tensor=sums.tensor, offset=sums.offset,
                                    ap=[sums.ap[0], [2, NGRP]]),
                        in1=bass.AP(tensor=sums.tensor,
                                    offset=sums.offset + 1,
                                    ap=[sums.ap[0], [2, NGRP]]),
                        op=ALU.add)
                    nc.vector.reciprocal(out=rcp4[:, 0:NGRP],
                                         in_=z4[:, 0:NGRP])
                    # ct = exp(cols); p = ct/Z (in place); q = 1 - p
                    nc.scalar.activation(out=ct4[:, :], in_=cols4[:, :],
                                         func=ACTF.Exp)
                    nc.vector.tensor_tensor(
                        out=ct4[:, :], in0=ct4[:, :],
                        in1=bass.AP(tensor=rcp4.tensor, offset=rcp4.offset,
                                    ap=[rcp4.ap[0], [1, NGRP], [0, C]]),
                        op=ALU.mult)
                    nc.gpsimd.tensor_scalar(
                        out=qt4[:, :], in0=ct4[:, :], scalar1=-1.0,
                        scalar2=1.0, op0=ALU.mult, op1=ALU.add)
                    # R scans per group: rext[g][0]=1, rext[g][1+j]=prod q
                    nc.gpsimd.memset(
                        bass.AP(tensor=rext4.tensor, offset=rext4.offset,
                                ap=[rext4.ap[0], [CE, NGRP], [1, 1]]), 1.0)
                    for g in range(NGRP):
                        nc.vector.tensor_tensor_scan(
                            out=rext4[:, g * CE + 1:(g + 1) * CE],
                            data0=qt4[:, g * C:(g + 1) * C],
                            data1=ones[:, :], initial=1.0,
                            op0=ALU.mult, op1=ALU.mult)
                    nc.vector.reciprocal(out=irext4[:, :], in_=rext4[:, :])
                    # pinv = p * (1/R)  (in place over ct4)
                    nc.gpsimd.tensor_tensor(
                        out=ct4[:, :], in0=ct4[:, :],
                        in1=bass.AP(tensor=irext4.tensor,
                                    offset=irext4.offset + 1,
                                    ap=[irext4.ap[0], [CE, NGRP], [1, C]]),
                        op=ALU.mult)
                    # rp[g][ph][1+j] = R row (g*P+ph-1); row -1 = ones
                    nc.gpsimd.memset(rp4[:, :], 1.0)
                    for g in range(NGRP):
                        nc.sync.dma_start(
                            out=rp4[1:P, g * CE + 1:(g + 1) * CE],
                            in_=rext4[0:P - 1, g * CE + 1:(g + 1) * CE])
                        if g:
                            nc.sync.dma_start(
                                out=rp4[0:1, g * CE + 1:(g + 1) * CE],
                                in_=rext4[P - 1:P,
                                          (g - 1) * CE + 1:g * CE])
                    # F_j = R'_{j-1}*pinv_j ; E_j = R'_j / R_{j-1}
                    nc.gpsimd.tensor_tensor(
                        out=bass.AP(tensor=fe8.tensor, offset=fe8.offset,
                                    ap=[fe8.ap[0], [2 * C, NGRP], [1, C]]),
                        in0=bass.AP(tensor=rp4.tensor, offset=rp4.offset,
                                    ap=[rp4.ap[0], [CE, NGRP], [1, C]]),
                        in1=bass.AP(tensor=ct4.tensor, offset=ct4.offset,
                                    ap=[ct4.ap[0], [C, NGRP], [1, C]]),
                        op=ALU.mult)
                    nc.gpsimd.tensor_tensor(
                        out=bass.AP(tensor=fe8.tensor, offset=fe8.offset + C,
                                    ap=[fe8.ap[0], [2 * C, NGRP], [1, C]]),
                        in0=bass.AP(tensor=rp4.tensor, offset=rp4.offset + 1,
                                    ap=[rp4.ap[0], [CE, NGRP], [1, C]]),
                        in1=bass.AP(tensor=irext4.tensor,
                                    offset=irext4.offset,
                                    ap=[irext4.ap[0], [CE, NGRP], [1, C]]),
                        op=ALU.mult)
                    # interleave pinv into d1 images in SBUF (odd slots) so
                    # the DRAM pack below is fully contiguous (a stride-2
                    # DRAM scatter would cost ~65k 4B descriptors)
                    nc.vector.memset(d1img[:, :], 0.0)
                    nc.gpsimd.tensor_scalar(
                        out=bass.AP(tensor=d1img.tensor,
                                    offset=d1img.offset + 1,
                                    ap=[d1img.ap[0], [2 * C, NGRP], [2, C]]),
                        in0=bass.AP(tensor=ct4.tensor, offset=ct4.offset,
                                    ap=[ct4.ap[0], [C, NGRP], [1, C]]),
                        scalar1=1.0, scalar2=None, op0=ALU.mult)
                    # pack rows: [0,1024) F,E ; [1024,2048) d1 (odd = pinv)
                    for g in range(NGRP):
                        poff = g * (P * DROW)
                        nc.sync.dma_start(
                            out=bass.AP(tensor=packed[:].tensor, offset=poff,
                                        ap=[[DROW, P], [1, 2 * FW]]),
                            in_=fe8[:, g * 2 * C:(g + 1) * 2 * C])
                        nc.sync.dma_start(
                            out=bass.AP(tensor=packed[:].tensor,
                                        offset=poff + 2 * FW,
                                        ap=[[DROW, P], [1, 2 * FW]]),
                            in_=d1img[:, g * 2 * C:(g + 1) * 2 * C])
                    # R_511 for the epilogue (group 3, partition 127, col C)
                    nc.sync.dma_start(
                        out=rlast[0:1, 0:1],
                        in_=rext4[P - 1:P, NGRP * CE - 1:NGRP * CE])

            def emit_phase2():
                with tc.tile_pool(name="dp", bufs=1) as dp:
                    # padded to 1040 so a1/slots keep a0's 64B phase and the
                    # +1-offset scan output stream sits 4B off the d0/d1
                    # read phase
                    a0 = dp.tile([1, 2 * C + 16], F32, tag="a0")
                    a1 = dp.tile([1, 2 * C + 16], F32, tag="a1")
                    nc.gpsimd.memset(a0[:, :], 0.0)
                    nc.gpsimd.memset(a1[:, :], 0.0)
                    abufs = [a0, a1]
                    slots = dp.tile([1, 2 * U * SLOT], F32, tag="slots",
                                    name="slots")
                    # d0 odd slots = -BIG, persistent scratch
                    nc.gpsimd.memset(
                        bass.AP(tensor=slots.tensor,
                                offset=slots.offset + 4 * FW + 1,
                                ap=[slots.ap[0], [SLOT, 2 * U], [2, FW]]),
                        NEGBIG)

                    def stage_dma(half, src_off):
                        dst = bass.AP(
                            tensor=slots.tensor,
                            offset=slots.offset + half * U * SLOT,
                            ap=[slots.ap[0], [SLOT, U], [1, DROW]])
                        src = bass.AP(tensor=packed[:].tensor,
                                      offset=src_off,
                                      ap=[[1, U * DROW]])
                        nc.sync.dma_start(out=dst, in_=src)

                    def row(half, k):
                        so = slots.offset + (half * U + k) * SLOT
                        aprev = abufs[k % 2]
                        acur = abufs[(k + 1) % 2]
                        ain = bass.AP(
                            tensor=aprev.tensor, offset=aprev.offset,
                            ap=[aprev.ap[0], [2, 2], [2, C]])
                        fe = bass.AP(
                            tensor=slots.tensor, offset=so,
                            ap=[slots.ap[0], [FW, 2], [1, C]])
                        vkout = bass.AP(
                            tensor=slots.tensor, offset=so + 2 * FW,
                            ap=[slots.ap[0], [2 * FW, 2], [2, C]])
                        nc.vector.tensor_tensor(
                            out=vkout, in0=ain, in1=fe, op=ALU.mult)
                        d1 = bass.AP(tensor=slots.tensor, offset=so + 2 * FW,
                                     ap=[slots.ap[0], [1, 2 * C]])
                        d0 = bass.AP(tensor=slots.tensor, offset=so + 4 * FW,
                                     ap=[slots.ap[0], [1, 2 * C]])
                        nc.vector.tensor_tensor_scan(
                            out=acur[0:1, 1:2 * C + 1], data0=d0,
                            data1=d1, initial=0.0,
                            op0=ALU.max, op1=ALU.add)

                    # prologue: rows [0, U) -> half 0
                    stage_dma(0, 0)
                    with tc.For_i(0, NIT) as it:
                        base = it * (2 * U * DROW)
                        stage_dma(1, base + U * DROW)
                        for k in range(U):
                            row(0, k)
                        stage_dma(0, base + 2 * U * DROW)
                        for k in range(U):
                            row(1, k)
                    # epilogue: D = alpha_511 * R_511
                    final = abufs[0]
                    dres = dp.tile([1, 1], F32, tag="dres")
                    nc.gpsimd.tensor_tensor(
                        out=dres[:, :], in0=final[0:1, 2 * C:2 * C + 1],
                        in1=rlast[0:1, 0:1], op=ALU.mult)
                    nc.sync.dma_start(out=out[:, :], in_=dres[:, :])

            for _rep in range(max(1, timing_reps)):
                if _rep:
                    tc.strict_bb_all_engine_barrier()
                if do_phase1:
                    emit_phase1()
                else:
                    nc.gpsimd.memset(rlast[:, :], 1.0)
                if do_dp:
                    emit_phase2()
                else:
                    nc.sync.dma_start(out=out[:, :], in_=rlast[0:1, 0:1])

    _split_multi_waits(nc)
    return nc


def kernel(batch: np.ndarray, docs: np.ndarray) -> np.ndarray:
    batch = np.ascontiguousarray(np.asarray(batch, dtype=np.float32))
    docs = np.asarray(docs)
    assert batch.shape == (B, R, V) and docs.shape == (B, C)

    import ml_dtypes
    nc = build_nc()
    in_maps = []
    xbf = batch.astype(ml_dtypes.bfloat16)
    for b in range(B):
        cols_b = np.ascontiguousarray(batch[b][:, docs[b].astype(np.int64)])
        in_maps.append({"x": xbf[b], "cols": cols_b})

    res = bass_utils.run_bass_kernel_spmd(nc, in_maps, core_ids=list(range(B)))
    d_vals = np.array(
        [res.results[b]["out"][0, 0] for b in range(B)], dtype=np.float64
    )
    d_vals = np.minimum(d_vals, CLAMP)
    loss = -np.log(d_vals.mean() / float(C))
    return np.float32(loss)



# revision 3
# speedup vs baseline: 7.9554x; 7.9554x over previous
"""CaLCS (soft-LCS) loss kernel for Trainium2, 8 NeuronCores, data-parallel
over batch — SPLIT into two device programs.

Program 1 (phase 1, loop-free): stream logits (exp + row-sum on ACT),
build per-row scan constants, pack DRAM rows of 2048 f32:
[F(512), E(512), d1(1024, odd=pinv)]. Outputs `packed` + `rlast`.

Program 2 (phase 2, one For_i hardware loop): the serial DP chain.
Per row: one product tensor_tensor (writes d1 even = khat, d0 even = v)
and one 1024-elem interleaved (max,+) scan, both DVE. Two-half
double-buffered staging DMA from `packed`.

Why split: on this part, a program that contains BOTH the unrolled
phase-1 instruction stream (ACT ops / wide DMAs) AND a For_i hardware
loop runs ~2x slower than the two pieces run as separate programs
(measured: fused ~10ms vs split ~1.1ms + ~4.3ms). The split costs only
host round-trip wall time, which is off the device-time metric.

Math (per core, one example; probs = softmax over V=32000):
  p[i,j] = probs[i, docs[j]] (cols pre-gathered on host), q = 1-p
  D[i,j] = p*(1+D[i-1,j-1]) + q*max(D[i-1,j], D[i,j-1])
  loss   = -log(mean_b min(D[511,511], 100) / 512)
Normalized alpha_j = D[i,j]/R_j with R_j = prod_{t<=j} q_t turns the row
recurrence into an interleaved (max,+) scan:
  even step: s = max(v_j, s) + khat_j ; odd step: s = s + pinv_j
with khat_j = alpha'_{j-1}*F_j, v_j = alpha'_j*E_j,
F_j = R'_{j-1}*pinv_j, E_j = R'_j/R_{j-1}, pinv_j = p_j/R_j.
Host: gathers 8 clamped D values, returns -log(mean/512).
"""

import numpy as np

import bass_rust
import concourse.bass as bass
import concourse.tile as tile
import concourse.mybir as mybir
from concourse import bass_utils

# ---- problem constants (hardcoded per contract) ----
B = 8
R = 512          # generation steps (rows of DP grid)
V = 32000        # vocab
C = 512          # doc length (cols of DP grid)
CLAMP = 100.0
P = 128          # SBUF partitions
NGRP = R // P    # 4 row groups
FW = C
DROW = 4 * FW    # packed DRAM row: F, E, d1(1024)  = 2048 f32
SLOT = 6 * FW    # SBUF slot: F, E, d1(1024), d0(1024) = 3072 f32
U = 8            # rows per half-body; body = 2U rows
NIT = R // (2 * U)
CE = C + 1       # R-vector row width (with leading 1)
NEGBIG = -1.0e30
NPACK = (R + 2 * U) * DROW

F32 = mybir.dt.float32
BF16 = mybir.dt.bfloat16
ALU = mybir.AluOpType
ACTF = mybir.ActivationFunctionType


def _patched_drain_and_barrier(self, tick_clock, wait_clock):
    """Split the kernel-tail drain's sem waits across multiple drain
    instructions — core_v3 codegen rejects multi-wait CTRL instructions."""
    from concourse.tile import ScopedClock

    nc = self.nc
    probe = nc.sync.drain()
    wait_clock.add_sem_waits(probe.ins, ScopedClock({None: tick_clock.global_clock}))
    waits = list(probe.ins.sync_info.on_wait) if probe.ins.sync_info else []
    if len(waits) > 1:
        probe.ins.sync_info = bass_rust.SyncInfo(on_wait=waits[:1], on_update=[])
        for i in range(1, len(waits)):
            d = nc.sync.drain()
            d.ins.sync_info = bass_rust.SyncInfo(on_wait=[waits[i]], on_update=[])
    nc.all_engine_barrier()
    popped = nc._tile_sem_poison_stack.pop()
    assert popped is self._sem_poison
    nc.clear_and_free_semaphores(list(self.sems.allocated().values()))
    nc.all_engine_barrier()


tile.TileContext._drain_and_barrier = _patched_drain_and_barrier


def _split_multi_waits(nc: bass.Bass):
    """Walrus codegen for TRN2 accepts at most one sem wait per instruction.
    Hoist extra waits into same-engine NoOp/Drain instructions inserted
    immediately before the offending instruction."""
    n_split = 0
    for fn in nc.m.functions:
        for blk in fn.blocks:
            il = blk.instructions
            i = 0
            while i < len(il):
                inst = il[i]
                si = inst.sync_info
                if si is not None and len(si.on_wait) > 1:
                    waits = list(si.on_wait)
                    inst.sync_info = bass_rust.SyncInfo(
                        on_wait=[waits[0]], on_update=list(si.on_update)
                    )
                    for k, w in enumerate(waits[1:]):
                        if inst.engine == mybir.EngineType.PE:
                            filler = mybir.InstDrain(
                                name=f"wsplit-{inst.name}-{k}", engine=inst.engine,
                                sync_info=bass_rust.SyncInfo(on_wait=[w], on_update=[]),
                            )
                        else:
                            filler = mybir.InstNoOp(
                                name=f"wsplit-{inst.name}-{k}", engine=inst.engine,
                                sync_info=bass_rust.SyncInfo(on_wait=[w], on_update=[]),
                            )
                        il.insert(i, filler)
                        i += 1
                        n_split += 1
                i += 1
    return n_split


def build_p1(timing_reps: int = 0) -> bass.Bass:
    """Phase-1 program: softmax sums + scan-constant packing. No loops."""
    nc = bass.Bass(trn_type="TRN2")
    kind = "Internal" if timing_reps else "ExternalInput"
    x = nc.dram_tensor("x", [R, V], BF16, kind=kind)
    cols = nc.dram_tensor("cols", [R, C], F32, kind=kind)
    packed = nc.dram_tensor("packed", [NPACK], F32, kind="ExternalOutput")
    rlast_d = nc.dram_tensor("rlast", [1, 1], F32, kind="ExternalOutput")

    with tile.TileContext(nc) as tc:
        if timing_reps:
            with tc.tile_pool(name="zpool", bufs=1) as zpool:
                zx = zpool.tile([P, V], BF16, tag="zx")
                nc.vector.memset(zx[:, :], 0.0)
                zxf = zpool.tile([P, C], F32, tag="zxf")
                nc.vector.memset(zxf[:, :], 0.0)
                for g in range(NGRP):
                    nc.gpsimd.dma_start(
                        out=x[g * P:(g + 1) * P, :], in_=zx[:, :])
                    nc.gpsimd.dma_start(
                        out=cols[g * P:(g + 1) * P, :], in_=zxf[:, :])
            tc.strict_bb_all_engine_barrier()

        for _rep in range(max(1, timing_reps)):
            if _rep:
                tc.strict_bb_all_engine_barrier()
            with (
                tc.tile_pool(name="chunks", bufs=1) as chunks,
                tc.tile_pool(name="p1", bufs=1) as p1,
            ):
                ones = p1.tile([P, C], F32, tag="ones")
                nc.gpsimd.memset(ones[:, :], 1.0)
                cols4 = p1.tile([P, NGRP * C], F32, tag="cols4")
                ct4 = p1.tile([P, NGRP * C], F32, tag="ct4")
                qt4 = p1.tile([P, NGRP * C], F32, tag="qt4")
                fe8 = p1.tile([P, NGRP * 2 * C], F32, tag="fe8")
                sums = p1.tile([P, 16], F32, tag="sums")
                rcp4 = p1.tile([P, 16], F32, tag="rcp4")
                d1img = p1.tile([P, NGRP * 2 * C], F32, tag="d1img")
                rext4 = p1.tile([P, NGRP * CE], F32, tag="rext4")
                irext4 = p1.tile([P, NGRP * CE], F32, tag="irext4")
                rp4 = p1.tile([P, NGRP * CE], F32, tag="rp4")

                # cols into SBUF (static, SP queue)
                for g in range(NGRP):
                    nc.sync.dma_start(
                        out=cols4[:, g * C:(g + 1) * C],
                        in_=cols[g * P:(g + 1) * P, :])
                # exp+row-sum stream: one whole-group chunk per ACT op
                for g in range(NGRP):
                    t = chunks.tile([P, V], BF16, tag="stream")
                    nc.gpsimd.dma_start(
                        out=t[:, :], in_=x[g * P:(g + 1) * P, :])
                    nc.scalar.activation(
                        out=t[:, :], in_=t[:, :], func=ACTF.Exp,
                        accum_out=sums[:, g:g + 1])
                nc.vector.reciprocal(out=rcp4[:, 0:NGRP],
                                     in_=sums[:, 0:NGRP])
                # ct = exp(cols); p = ct/Z (in place); q = 1 - p
                nc.scalar.activation(out=ct4[:, :], in_=cols4[:, :],
                                     func=ACTF.Exp)
                nc.vector.tensor_tensor(
                    out=ct4[:, :], in0=ct4[:, :],
                    in1=bass.AP(tensor=rcp4.tensor, offset=rcp4.offset,
                                ap=[rcp4.ap[0], [1, NGRP], [0, C]]),
                    op=ALU.mult)
                nc.gpsimd.tensor_scalar(
                    out=qt4[:, :], in0=ct4[:, :], scalar1=-1.0,
                    scalar2=1.0, op0=ALU.mult, op1=ALU.add)
                # R scans per group: rext[g][0]=1, rext[g][1+j]=prod q
                nc.gpsimd.memset(
                    bass.AP(tensor=rext4.tensor, offset=rext4.offset,
                            ap=[rext4.ap[0], [CE, NGRP], [1, 1]]), 1.0)
                for g in range(NGRP):
                    nc.vector.tensor_tensor_scan(
                        out=rext4[:, g * CE + 1:(g + 1) * CE],
                        data0=qt4[:, g * C:(g + 1) * C],
                        data1=ones[:, :], initial=1.0,
                        op0=ALU.mult, op1=ALU.mult)
                nc.vector.reciprocal(out=irext4[:, :], in_=rext4[:, :])
                # pinv = p * (1/R)  (in place over ct4)
                nc.gpsimd.tensor_tensor(
                    out=ct4[:, :], in0=ct4[:, :],
                    in1=bass.AP(tensor=irext4.tensor,
                                offset=irext4.offset + 1,
                                ap=[irext4.ap[0], [CE, NGRP], [1, C]]),
                    op=ALU.mult)
                # rp[g][ph][1+j] = R row (g*P+ph-1); row -1 = ones
                nc.gpsimd.memset(rp4[:, :], 1.0)
                for g in range(NGRP):
                    nc.sync.dma_start(
                        out=rp4[1:P, g * CE + 1:(g + 1) * CE],
                        in_=rext4[0:P - 1, g * CE + 1:(g + 1) * CE])
                    if g:
                        nc.sync.dma_start(
                            out=rp4[0:1, g * CE + 1:(g + 1) * CE],
                            in_=rext4[P - 1:P, (g - 1) * CE + 1:g * CE])
                # F_j = R'_{j-1}*pinv_j ; E_j = R'_j / R_{j-1}
                nc.gpsimd.tensor_tensor(
                    out=bass.AP(tensor=fe8.tensor, offset=fe8.offset,
                                ap=[fe8.ap[0], [2 * C, NGRP], [1, C]]),
                    in0=bass.AP(tensor=rp4.tensor, offset=rp4.offset,
                                ap=[rp4.ap[0], [CE, NGRP], [1, C]]),
                    in1=bass.AP(tensor=ct4.tensor, offset=ct4.offset,
                                ap=[ct4.ap[0], [C, NGRP], [1, C]]),
                    op=ALU.mult)
                nc.gpsimd.tensor_tensor(
                    out=bass.AP(tensor=fe8.tensor, offset=fe8.offset + C,
                                ap=[fe8.ap[0], [2 * C, NGRP], [1, C]]),
                    in0=bass.AP(tensor=rp4.tensor, offset=rp4.offset + 1,
                                ap=[rp4.ap[0], [CE, NGRP], [1, C]]),
                    in1=bass.AP(tensor=irext4.tensor,
                                offset=irext4.offset,
                                ap=[irext4.ap[0], [CE, NGRP], [1, C]]),
                    op=ALU.mult)
                # interleave pinv into d1 images (odd slots) so the DRAM
                # pack below is fully contiguous
                nc.vector.memset(d1img[:, :], 0.0)
                nc.gpsimd.tensor_scalar(
                    out=bass.AP(tensor=d1img.tensor,
                                offset=d1img.offset + 1,
                                ap=[d1img.ap[0], [2 * C, NGRP], [2, C]]),
                    in0=bass.AP(tensor=ct4.tensor, offset=ct4.offset,
                                ap=[ct4.ap[0], [C, NGRP], [1, C]]),
                    scalar1=1.0, scalar2=None, op0=ALU.mult)
                # pack rows: [0,1024) F,E ; [1024,2048) d1 (odd = pinv)
                for g in range(NGRP):
                    poff = g * (P * DROW)
                    nc.sync.dma_start(
                        out=bass.AP(tensor=packed[:].tensor, offset=poff,
                                    ap=[[DROW, P], [1, 2 * FW]]),
                        in_=fe8[:, g * 2 * C:(g + 1) * 2 * C])
                    nc.sync.dma_start(
                        out=bass.AP(tensor=packed[:].tensor,
                                    offset=poff + 2 * FW,
                                    ap=[[DROW, P], [1, 2 * FW]]),
                        in_=d1img[:, g * 2 * C:(g + 1) * 2 * C])
                # R_511 for the epilogue
                nc.sync.dma_start(
                    out=rlast_d[:, :],
                    in_=rext4[P - 1:P, NGRP * CE - 1:NGRP * CE])

    _split_multi_waits(nc)
    return nc


def build_p2(timing_reps: int = 0) -> bass.Bass:
    """Phase-2 program: the serial DP chain (For_i loop only)."""
    nc = bass.Bass(trn_type="TRN2")
    kind = "Internal" if timing_reps else "ExternalInput"
    packed = nc.dram_tensor("packed", [NPACK], F32, kind=kind)
    rlast_d = nc.dram_tensor("rlast", [1, 1], F32, kind=kind)
    out = nc.dram_tensor("out", [1, 1], F32, kind="ExternalOutput")

    with tile.TileContext(nc) as tc:
        with tc.tile_pool(name="keep", bufs=1) as keep:
            rlast = keep.tile([1, 64], F32, tag="rlast")
            if timing_reps:
                with tc.tile_pool(name="zpool", bufs=1) as zpool:
                    zr = zpool.tile([1, U * DROW], F32, tag="zr")
                    nc.vector.memset(zr[:, :], 0.0)
                    for r in range(0, NPACK, U * DROW):
                        nc.gpsimd.dma_start(
                            out=bass.AP(tensor=packed[:].tensor, offset=r,
                                        ap=[[1, U * DROW]]),
                            in_=zr[0:1, :])
                    zf = zpool.tile([1, 16], F32, tag="zf")
                    nc.vector.memset(zf[:, :], 1.0)
                    nc.gpsimd.dma_start(out=rlast_d[:, :], in_=zf[0:1, 0:1])
                tc.strict_bb_all_engine_barrier()

            for _rep in range(max(1, timing_reps)):
                if _rep:
                    tc.strict_bb_all_engine_barrier()
                nc.sync.dma_start(out=rlast[0:1, 0:1], in_=rlast_d[:, :])
                with tc.tile_pool(name="dp", bufs=1) as dp:
                    # padded to 1040 so a1/slots keep a0's 64B phase and the
                    # +1-offset scan output stream sits 4B off the d0/d1
                    # read phase
                    a0 = dp.tile([1, 2 * C + 16], F32, tag="a0")
                    a1 = dp.tile([1, 2 * C + 16], F32, tag="a1")
                    nc.gpsimd.memset(a0[:, :], 0.0)
                    nc.gpsimd.memset(a1[:, :], 0.0)
                    abufs = [a0, a1]
                    slots = dp.tile([1, 2 * U * SLOT], F32, tag="slots")
                    # d0 odd slots = -BIG, persistent scratch
                    nc.gpsimd.memset(
                        bass.AP(tensor=slots.tensor,
                                offset=slots.offset + 4 * FW + 1,
                                ap=[slots.ap[0], [SLOT, 2 * U], [2, FW]]),
                        NEGBIG)

                    def stage_dma(half, src_off):
                        dst = bass.AP(
                            tensor=slots.tensor,
                            offset=slots.offset + half * U * SLOT,
                            ap=[slots.ap[0], [SLOT, U], [1, DROW]])
                        src = bass.AP(tensor=packed[:].tensor,
                                      offset=src_off,
                                      ap=[[1, U * DROW]])
                        nc.sync.dma_start(out=dst, in_=src)

                    def row(half, k):
                        so = slots.offset + (half * U + k) * SLOT
                        aprev = abufs[k % 2]
                        acur = abufs[(k + 1) % 2]
                        ain = bass.AP(
                            tensor=aprev.tensor, offset=aprev.offset,
                            ap=[aprev.ap[0], [2, 2], [2, C]])
                        fe = bass.AP(
                            tensor=slots.tensor, offset=so,
                            ap=[slots.ap[0], [FW, 2], [1, C]])
                        vkout = bass.AP(
                            tensor=slots.tensor, offset=so + 2 * FW,
                            ap=[slots.ap[0], [2 * FW, 2], [2, C]])
                        nc.vector.tensor_tensor(
                            out=vkout, in0=ain, in1=fe, op=ALU.mult)
                        d1 = bass.AP(tensor=slots.tensor, offset=so + 2 * FW,
                                     ap=[slots.ap[0], [1, 2 * C]])
                        d0 = bass.AP(tensor=slots.tensor, offset=so + 4 * FW,
                                     ap=[slots.ap[0], [1, 2 * C]])
                        nc.vector.tensor_tensor_scan(
                            out=acur[0:1, 1:2 * C + 1], data0=d0,
                            data1=d1, initial=0.0,
                            op0=ALU.max, op1=ALU.add)

                    # prologue: rows [0, U) -> half 0
                    stage_dma(0, 0)
                    with tc.For_i(0, NIT) as it:
                        base = it * (2 * U * DROW)
                        stage_dma(1, base + U * DROW)
                        for k in range(U):
                            row(0, k)
                        stage_dma(0, base + 2 * U * DROW)
                        for k in range(U):
                            row(1, k)
                    # epilogue: D = alpha_511 * R_511
                    final = abufs[0]
                    dres = dp.tile([1, 1], F32, tag="dres")
                    nc.gpsimd.tensor_tensor(
                        out=dres[:, :], in0=final[0:1, 2 * C:2 * C + 1],
                        in1=rlast[0:1, 0:1], op=ALU.mult)
                    nc.sync.dma_start(out=out[:, :], in_=dres[:, :])

    _split_multi_waits(nc)
    return nc


def kernel(batch: np.ndarray, docs: np.ndarray) -> np.ndarray:
    import ml_dtypes
    batch = np.ascontiguousarray(np.asarray(batch, dtype=np.float32))
    docs = np.asarray(docs)
    assert batch.shape == (B, R, V) and docs.shape == (B, C)

    xbf = batch.astype(ml_dtypes.bfloat16)
    in_maps = []
    for b in range(B):
        cols_b = np.ascontiguousarray(batch[b][:, docs[b].astype(np.int64)])
        in_maps.append({"x": xbf[b], "cols": cols_b})

    nc1 = build_p1()
    res1 = bass_utils.run_bass_kernel_spmd(nc1, in_maps,
                                           core_ids=list(range(B)))
    nc2 = build_p2()
    in2 = [{"packed": res1.results[b]["packed"],
            "rlast": res1.results[b]["rlast"]} for b in range(B)]
    res2 = bass_utils.run_bass_kernel_spmd(nc2, in2,
                                           core_ids=list(range(B)))
    d_vals = np.array([res2.results[b]["out"][0, 0] for b in range(B)],
                      dtype=np.float64)
    d_vals = np.minimum(d_vals, CLAMP)
    loss = -np.log(d_vals.mean() / float(C))
    return np.float32(loss)
